# revision 2
# baseline (speedup 1.0000x reference)
"""Trainium2 Bass kernel for nn_EdgeDecoder_lgcn (gnn_message_passing).

Computation (reference):
    logit = tanh(z_src @ W1 + b1) @ w2            # [NS]
    beta  = softmax(where(mask, logit, -inf), 1)  # [G, NS]
    agg   = beta @ z_src                          # [G, H]
    scores= agg @ z_dst.T                         # [G, ND]

Sharding: NS is split across the 8 cores for phase 1 (each core computes
logits for its slice and the partial masked-exp sums U_part = w.T@[z|1]
with w[i,g] = mask[g,i]*exp(logit[i])), a 66 KB AllReduce combines
(U, s), and ND is split across the cores for phase 2
(scores_slice = (U/s) @ z_dst_slice.T).

No max-subtraction is needed in the softmax: logit ~ N(0, 0.62), so
exp(logit) is far from overflow and fp32 exp/sums match the reference
to ~1e-6.
"""

import numpy as np

NS = 50000
ND = 50000
G = 128
H = 128
NCORES = 8
TPD = 49                 # 128-row i-tiles per device
NSL = TPD * 128          # 6272 rows per device slice
NSP = NCORES * NSL       # 50176 padded NS
NDL = NSL
NDP = NSP
GRP = 4                  # i-tiles batched per 512-wide group
NGRP = (TPD + GRP - 1) // GRP

# dtype knobs for the three big matmul groups (fp32 = exact, fp32r = fast)
T_MM_F32R = True        # t = z @ W1          (N=512 moving)
SC_MM_F32R = True       # scores = U @ zdT    (N=512 moving)
U_MM_F32R = False       # U accumulate        (N=129 moving: no speedup)

_CACHE = {}


def _build_module():
    import concourse.bacc as bacc
    import concourse.mybir as mybir
    import concourse.tile as tile
    from concourse import masks

    fp32 = mybir.dt.float32
    fp32r = mybir.dt.float32r
    i32 = mybir.dt.int32

    t_dt = fp32r if T_MM_F32R else fp32
    sc_dt = fp32r if SC_MM_F32R else fp32

    nc = bacc.Bacc(
        "TRN2", target_bir_lowering=False, debug=False, num_devices=NCORES
    )

    zs = nc.dram_tensor("zs", [NSL, H], fp32, kind="ExternalInput").ap()
    sym = nc.dram_tensor("sym", [G, NSL], i32, kind="ExternalInput").ap()
    zd = nc.dram_tensor("zd", [NDL, H], fp32, kind="ExternalInput").ap()
    W1 = nc.dram_tensor("W1", [H, H], fp32, kind="ExternalInput").ap()
    b1 = nc.dram_tensor("b1", [H, 1], fp32, kind="ExternalInput").ap()
    w2 = nc.dram_tensor("w2", [H, 1], fp32, kind="ExternalInput").ap()
    out = nc.dram_tensor("scores", [G, NDL], fp32, kind="ExternalOutput").ap()

    cc_in = nc.dram_tensor("cc_in", [G, H + 1], fp32)
    cc_out = nc.dram_tensor("cc_out", [G, H + 1], fp32, addr_space="Shared")

    Tanh = mybir.ActivationFunctionType.Tanh
    Exp = mybir.ActivationFunctionType.Exp

    with tile.TileContext(nc) as tc:
        with (
            tc.tile_pool(name="const", bufs=1) as cpool,
            tc.tile_pool(name="big", bufs=1) as big,
            tc.tile_pool(name="sbA", bufs=3) as sbA,
            tc.tile_pool(name="sbB", bufs=3) as sbB,
            tc.tile_pool(name="sbC", bufs=1) as sbC,
            tc.tile_pool(name="sbD", bufs=3) as sbD,
            tc.tile_pool(name="zt_ps", bufs=2, space="PSUM") as ztp,
            tc.tile_pool(name="t_ps", bufs=2, space="PSUM") as ttp,
            tc.tile_pool(name="lg_ps", bufs=1, space="PSUM") as lgp,
            tc.tile_pool(name="mt_ps", bufs=2, space="PSUM") as mtp,
            tc.tile_pool(name="u_ps", bufs=1, space="PSUM") as upl,
        ):
            # ---- constants ----
            W1_sb = cpool.tile([H, H], fp32)          # [h, h'] natural
            nc.sync.dma_start(out=W1_sb[:], in_=W1)
            W1t_sb = cpool.tile([H, H], t_dt)
            nc.scalar.copy(W1t_sb[:], W1_sb[:])
            b1_sb = cpool.tile([H, 1], fp32)
            nc.sync.dma_start(out=b1_sb[:], in_=b1)
            w2_sb = cpool.tile([H, 1], fp32)
            nc.sync.dma_start(out=w2_sb[:], in_=w2)
            ones_sb = cpool.tile([H, 1], fp32)
            nc.vector.memset(ones_sb[:], 1.0)
            ident = cpool.tile([128, 128], fp32)
            masks.make_identity(nc, ident[:])

            # ---- bulk inputs (chunked so compute can start early) ----
            # Zs1: partition p holds rows i = 49p + c, c in [0,49), each row
            # followed by a literal 1.0 -> tile c is [:, 129c : 129c+129]
            # = [z_i | 1], giving U and s from one matmul.
            Zs1_sb = big.tile([128, TPD * 129], fp32)
            Zs1v = Zs1_sb[:].rearrange("p (n x) -> p n x", x=129)
            zsv = zs.rearrange("(p n) h -> p n h", p=128)
            Ms_i32 = big.tile([128, NSL], i32)
            Ms_sb = big.tile([128, NSL], fp32)
            # mask col i = 49j + c  ->  [g, j, c] view, c innermost
            Msv = Ms_sb[:].rearrange("g (j c) -> g j c", c=TPD)
            Zd_sb = big.tile([128, NSL], fp32)
            Zdv = Zd_sb[:].rearrange("p (n h) -> p n h", h=128)
            zdv = zd.rearrange("(n p) h -> p n h", p=128)

            bounds = [0, 13, 26, 38, TPD]
            for k in range(4):
                lo, hi = bounds[k], bounds[k + 1]
                nc.sync.dma_start(
                    out=Zs1v[:, lo:hi, 0:128], in_=zsv[:, lo:hi, :]
                )
                nc.any.memset(Zs1v[:, lo:hi, 128:129], 1.0)
                nc.sync.dma_start(
                    out=Ms_i32[:, lo * 128 : hi * 128],
                    in_=sym[:, lo * 128 : hi * 128],
                )
                nc.vector.tensor_copy(
                    Ms_sb[:, lo * 128 : hi * 128],
                    Ms_i32[:, lo * 128 : hi * 128],
                )
                nc.sync.dma_start(out=Zdv[:, lo:hi, :], in_=zdv[:, lo:hi, :])

            e_sb = cpool.tile([128, TPD], fp32)

            # ---- pass A: logits, and pass B: U/s accumulation ----
            U_ps = upl.tile([G, H + 1], fp32)
            for g in range(NGRP):
                tiles = list(range(g * GRP, min((g + 1) * GRP, TPD)))
                W = len(tiles) * 128
                zT_ps = ztp.tile([128, GRP * 128], fp32, tag="zt")
                for j, c in enumerate(tiles):
                    nc.tensor.transpose(
                        zT_ps[:, j * 128 : (j + 1) * 128],
                        Zs1_sb[:, c * 129 : c * 129 + 128],
                        ident[:],
                    )
                zT_sb = sbA.tile([128, GRP * 128], t_dt, tag="zts")
                nc.scalar.copy(zT_sb[:, :W], zT_ps[:, :W])
                t_ps = ttp.tile([128, GRP * 128], fp32, tag="tps")
                nc.tensor.matmul(
                    t_ps[:, :W], W1t_sb[:], zT_sb[:, :W], start=True, stop=True
                )
                tanh_sb = sbA.tile([128, GRP * 128], fp32, tag="tanh")
                nc.scalar.activation(
                    tanh_sb[:, :W], t_ps[:, :W], Tanh, bias=b1_sb[:], scale=1.0
                )
                q_sb = sbA.tile([128, GRP * 128], fp32, tag="q")
                nc.scalar.mul(q_sb[:, :W], tanh_sb[:, :W], w2_sb[:])
                lg_ps = lgp.tile([128, GRP], fp32, tag="lg")
                for j, c in enumerate(tiles):
                    nc.tensor.matmul(
                        lg_ps[:, j : j + 1],
                        q_sb[:, j * 128 : (j + 1) * 128],
                        ones_sb[:],
                        start=True,
                        stop=True,
                    )
                nc.scalar.activation(
                    e_sb[:, tiles[0] : tiles[0] + len(tiles)],
                    lg_ps[:, : len(tiles)],
                    Exp,
                )
                # pass B for this group's tiles
                for j, c in enumerate(tiles):
                    mT_ps = mtp.tile([128, 128], fp32, tag="mt")
                    nc.tensor.transpose(mT_ps[:], Msv[:, :, c], ident[:])
                    w_sb = sbB.tile([128, 128], fp32, tag="w")
                    nc.scalar.mul(w_sb[:], mT_ps[:], e_sb[:, c : c + 1])
                    nc.tensor.matmul(
                        U_ps[:],
                        w_sb[:],
                        Zs1_sb[:, c * 129 : (c + 1) * 129],
                        start=(c == 0),
                        stop=(c == TPD - 1),
                    )

            # ---- pass C: AllReduce (U, s) and prep (U^T, 1/s) ----
            Us_sb = sbC.tile([G, H + 1], fp32)
            nc.scalar.copy(Us_sb[:], U_ps[:])
            nc.sync.dma_start(out=cc_in.ap(), in_=Us_sb[:])
            nc.gpsimd.collective_compute(
                "AllReduce",
                mybir.AluOpType.add,
                replica_groups=[list(range(NCORES))],
                ins=[cc_in.ap().opt()],
                outs=[cc_out.ap().opt()],
            )
            Usum_sb = sbC.tile([G, H + 1], fp32)
            nc.sync.dma_start(out=Usum_sb[:], in_=cc_out.ap())
            rs_sb = sbC.tile([G, 1], fp32)
            nc.vector.reciprocal(rs_sb[:], Usum_sb[:, H : H + 1])
            UT_ps = mtp.tile([128, 128], fp32, tag="mt")
            nc.tensor.transpose(UT_ps[:], Usum_sb[:, :H], ident[:])
            UT_sb = sbC.tile([H, G], sc_dt)
            nc.scalar.copy(UT_sb[:], UT_ps[:])

            # ---- pass D: scores slice ----
            for m in range(NGRP):
                tiles = list(range(m * GRP, min((m + 1) * GRP, TPD)))
                W = len(tiles) * 128
                zdT_ps = ztp.tile([128, GRP * 128], fp32, tag="zt")
                for j, n in enumerate(tiles):
                    nc.tensor.transpose(
                        zdT_ps[:, j * 128 : (j + 1) * 128],
                        Zd_sb[:, n * 128 : (n + 1) * 128],
                        ident[:],
                    )
                zdT_sb = sbD.tile([128, GRP * 128], sc_dt, tag="zdt")
                nc.scalar.copy(zdT_sb[:, :W], zdT_ps[:, :W])
                sc_ps = ttp.tile([G, GRP * 128], fp32, tag="tps")
                nc.tensor.matmul(
                    sc_ps[:, :W], UT_sb[:], zdT_sb[:, :W], start=True, stop=True
                )
                o_sb = sbD.tile([G, GRP * 128], fp32, tag="o")
                nc.vector.tensor_scalar_mul(o_sb[:, :W], sc_ps[:, :W], rs_sb[:])
                nc.sync.dma_start(
                    out=out[:, m * GRP * 128 : m * GRP * 128 + W],
                    in_=o_sb[:, :W],
                )

    nc.compile()
    return nc


def _get_module():
    if "nc" not in _CACHE:
        _CACHE["nc"] = _build_module()
    return _CACHE["nc"]


def kernel(z_src, z_dst, sym_indexs, W1, b1, w2):
    from concourse import bass_utils

    z_src = np.ascontiguousarray(np.asarray(z_src, dtype=np.float32))
    z_dst = np.ascontiguousarray(np.asarray(z_dst, dtype=np.float32))
    sym_indexs = np.ascontiguousarray(np.asarray(sym_indexs, dtype=np.int32))
    W1 = np.ascontiguousarray(np.asarray(W1, dtype=np.float32))
    b1 = np.ascontiguousarray(np.asarray(b1, dtype=np.float32)).reshape(H, 1)
    w2 = np.ascontiguousarray(np.asarray(w2, dtype=np.float32)).reshape(H, 1)

    ns, h = z_src.shape
    nd = z_dst.shape[0]
    g = sym_indexs.shape[0]
    assert (ns, nd, g, h) == (NS, ND, G, H), (ns, nd, g, h)

    zsp = np.zeros((NSP, H), dtype=np.float32)
    zsp[:NS] = z_src
    symp = np.zeros((G, NSP), dtype=np.int32)
    symp[:, :NS] = sym_indexs
    zdp = np.zeros((NDP, H), dtype=np.float32)
    zdp[:ND] = z_dst

    in_maps = []
    for k in range(NCORES):
        lo = k * NSL
        in_maps.append(
            {
                "zs": np.ascontiguousarray(zsp[lo : lo + NSL]),
                "sym": np.ascontiguousarray(symp[:, lo : lo + NSL]),
                "zd": np.ascontiguousarray(zdp[lo : lo + NSL]),
                "W1": W1,
                "b1": b1,
                "w2": w2,
            }
        )

    nc = _get_module()
    res = bass_utils.run_bass_kernel_spmd(
        nc, in_maps, core_ids=list(range(NCORES))
    )
    scores = np.empty((G, NDP), dtype=np.float32)
    for k in range(NCORES):
        scores[:, k * NDL : (k + 1) * NDL] = res.results[k]["scores"]
    return scores[:, :ND]


if __name__ == "__main__":
    rng = np.random.default_rng(0)
    inputs = {
        "z_src": rng.standard_normal((NS, H), dtype=np.float32),
        "z_dst": rng.standard_normal((ND, H), dtype=np.float32),
        "sym_indexs": rng.integers(0, 2, (G, NS), dtype=np.int32),
        "W1": rng.standard_normal((H, H), dtype=np.float32) / np.sqrt(H),
        "b1": np.zeros(H, dtype=np.float32),
        "w2": rng.standard_normal(H, dtype=np.float32) / np.sqrt(H),
    }
    out = kernel(**inputs)
    print(out.shape, out.dtype, np.abs(out).max())
